# revision 8
# baseline (speedup 1.0000x reference)
"""HSIC pairwise loss kernel for trn2 (8 NeuronCores).

Math: reference builds K_c = (w^2 w^2T) * (E_c E_c^T), M_c = R K_c, and sums
tr(M_i M_j) over i<j. With F_c = w^2 * E_c (row scaling), R the centering
matrix (idempotent):
    tr(R K_i R K_j) = ||G_i^T G_j||_F^2,  G_c = F_c - colmean(F_c)
and with A_ij = F_i^T F_j, s_c = F_c^T 1, t_c = F_c s_c:
    ||G_i^T G_j||_F^2 = ||A_ij||_F^2 - (2/n) t_i.t_j + ||s_i||^2 ||s_j||^2 / n^2

Device work: the 45 A_ij blocks [256,256] (contraction over n=4096) at
half-chunk granularity (20 units of 128 cols). Each core loads 9 units,
pre-scaled by w^2 and cast to fp8e4m3 on host, laid out as 16 k-macrotiles
of 256 rows ([128 partitions, 2 k-subtiles, 1152 cols]). Matmuls run in
fp8 DoubleRow perf mode (256-deep contraction per instruction, double PE
rate), accumulating the pair windows in PSUM over the 16 macrotiles.

Schedule details: the whole per-core input (4.7 MB) is SBUF-resident;
loads are paired (4.6 KB per partition line) and issued alternately from
the Sync and Activation DGE sequencers (descriptor generation is ~0.6us
per DMA and serializes per sequencer). The first macrotile is split so
the PE can start on a 160 KB prefix; dummy self-matmuls warm the PE out
of its low-power state during the initial DMA wait. Output: raw A blocks
cast bf16 (vector engine for the early-stopping windows, activation
engine for the tail) and written by one DMA per DGE sequencer. Host
computes the rank-1 centering corrections (s, t) exactly in float64 and
the final scalar reduction.
"""

import numpy as np
import ml_dtypes
from contextlib import ExitStack

import concourse.bass as bass
import concourse.tile as tile
from concourse import bacc, mybir
from concourse import bass_utils

N = 4096
MT = 16            # k macrotiles of 256 rows (2 DoubleRow subtiles of 128)
UNITS = 9          # half-chunk units per core
DCOLS = UNITS * 128          # 1152 data cols

# 8 cores x 9 ordered units (of 20 half-chunks); covers all 180
# cross-parent half-pairs via the window pattern below (found by search).
# Slot layout: slots 0-3 = "moving" block (cols 0:512), slots 4-8 =
# "stationary" block (cols 512:1152).
ASSIGN = [
    [18, 13, 17, 1, 0, 19, 9, 4, 3],
    [12, 9, 16, 15, 10, 5, 8, 18, 4],
    [2, 7, 14, 11, 5, 17, 12, 9, 18],
    [13, 1, 15, 16, 14, 7, 12, 11, 2],
    [6, 12, 11, 15, 0, 17, 3, 19, 4],
    [3, 19, 0, 6, 14, 2, 9, 16, 5],
    [6, 5, 8, 10, 17, 18, 13, 11, 1],
    [4, 7, 10, 8, 3, 2, 19, 0, 14],
]

# (stationary_col, moving_start_col, n_cols):
# bipartite {slots 4-8} x {slots 0-3} + clique on {0-3}
WINDOWS = [
    (512, 0, 512),
    (640, 0, 512),
    (768, 0, 512),
    (896, 0, 512),
    (1024, 0, 512),
    (0, 128, 384),
    (128, 256, 256),
    (256, 384, 128),
]
# cast engine per window: first VEC_W windows on vector, rest on scalar
VEC_W = 4
OUT_COLS = sum(w[2] for w in WINDOWS)
SPLIT_COL = sum(w[2] for w in WINDOWS[:VEC_W])
WARMUP_MM = 5      # dummy 512-col matmuls to ramp the PE p-state

_CACHE = {}


def _build():
    f32 = mybir.dt.float32
    bf16 = mybir.dt.bfloat16
    f8 = mybir.dt.float8e4
    DR = mybir.MatmulPerfMode.DoubleRow
    nc = bacc.Bacc("TRN2", target_bir_lowering=False, debug=False,
                   num_devices=8)
    # row 128*q + p holds [pair member j, k-subtile s, col c] for F row
    # 512*q + 256*j + 128*s + p
    x = nc.dram_tensor("x", [(MT // 2) * 128, 2, 2, DCOLS], f8,
                       kind="ExternalInput").ap()
    out = nc.dram_tensor("out", [128, OUT_COLS], bf16,
                         kind="ExternalOutput").ap()

    with tile.TileContext(nc) as tc:
        with ExitStack() as ctx:
            xpool = ctx.enter_context(tc.tile_pool(name="xs", bufs=1))
            psum = ctx.enter_context(tc.tile_pool(name="ps", bufs=1,
                                                  space="PSUM"))
            opool = ctx.enter_context(tc.tile_pool(name="o", bufs=1))

            # PSUM banks are 2KB (512 f32) and tiles are bank-granular:
            # pack the sub-512 windows into shared banks.
            ps = [None] * len(WINDOWS)   # (bank_tile, col_offset) per window
            bank_fill = []   # list of [tile, used_cols]
            for i, (_, _, nw) in enumerate(WINDOWS):
                placed = False
                for b in bank_fill:
                    if b[1] + nw <= 512:
                        ps[i] = (b[0], b[1])
                        b[1] += nw
                        placed = True
                        break
                if not placed:
                    t = psum.tile([128, max(nw, 512)], f32, tag=f"psb{i}",
                                  name=f"psb{i}")
                    ps[i] = (t, 0)
                    bank_fill.append([t, nw])

            # PE warmup: dummy DoubleRow matmuls (into window 0's bank,
            # before its accumulation group opens) keep the PE busy and
            # ramping to its fast p-state during the initial DMA wait.
            dsb = xpool.tile([128, 2, 512], f8, name="warm_in")
            nc.vector.memset(dsb[:], 0.0)
            for _ in range(WARMUP_MM):
                nc.tensor.matmul(ps[0][0][:, 0:512], dsb[:, :, 0:128],
                                 dsb[:, :, 0:512], start=True, stop=True,
                                 perf_mode=DR)

            # macrotile 0 split: 640-col prefix unblocks the first matmuls
            t0a1 = xpool.tile([128, 2, 640], f8, name="t0a1")
            t0a2 = xpool.tile([128, 2, 512], f8, name="t0a2")
            t0b = xpool.tile([128, 2, DCOLS], f8, name="t0b")
            nc.sync.dma_start(t0a1[:], x[0:128, 0:1, :, 0:640])
            nc.sync.dma_start(t0a2[:], x[0:128, 0:1, :, 640:DCOLS])
            nc.scalar.dma_start(t0b[:], x[0:128, 1:2, :, :])
            pair_tiles = []
            for q in range(1, MT // 2):
                tq = xpool.tile([128, 2, 2, DCOLS], f8, name=f"t{q}")
                eng = nc.sync if q % 2 == 0 else nc.scalar
                eng.dma_start(tq[:], x[q * 128:(q + 1) * 128, :, :, :])
                pair_tiles.append(tq)

            def slices(m, c0, c1):
                """AP for cols [c0:c1) of macrotile m."""
                if m == 0:
                    if c1 <= 640:
                        return t0a1[:, :, c0:c1]
                    return t0a2[:, :, c0 - 640:c1 - 640]
                if m == 1:
                    return t0b[:, :, c0:c1]
                return pair_tiles[m // 2 - 1][:, m % 2, :, c0:c1]

            # window order within a macrotile: for m=0, prefix-gated
            # windows first; for m=MT-1, vector-cast windows stop first
            worder_first = [0, 5, 6, 7, 1, 2, 3, 4]
            worder_first = [wi for wi in worder_first if wi < len(WINDOWS)]
            worder_mid = list(range(len(WINDOWS)))
            for m in range(MT):
                order = worder_first if m == 0 else worder_mid
                for wi in order:
                    stc, mc, nw = WINDOWS[wi]
                    pt, po = ps[wi]
                    nc.tensor.matmul(
                        pt[:, po:po + nw],
                        slices(m, stc, stc + 128),
                        slices(m, mc, mc + nw),
                        start=(m == 0),
                        stop=(m == MT - 1),
                        perf_mode=DR,
                    )

            ot = opool.tile([128, OUT_COLS], bf16)
            col = 0
            for wi, (s, mc, nw) in enumerate(WINDOWS):
                pt, po = ps[wi]
                if wi < VEC_W:
                    nc.vector.tensor_copy(ot[:, col:col + nw],
                                          pt[:, po:po + nw])
                else:
                    nc.scalar.copy(ot[:, col:col + nw], pt[:, po:po + nw])
                col += nw
            nc.sync.dma_start(out[:, 0:SPLIT_COL], ot[:, 0:SPLIT_COL])
            nc.scalar.dma_start(out[:, SPLIT_COL:OUT_COLS],
                                ot[:, SPLIT_COL:OUT_COLS])
    nc.compile()
    return nc


def _get_nc():
    if "nc" not in _CACHE:
        _CACHE["nc"] = _build()
    return _CACHE["nc"]


def _in_maps(F32):
    maps = []
    for units in ASSIGN:
        xc = np.concatenate([F32[:, u * 128:(u + 1) * 128] for u in units],
                            axis=1)
        # [4096, 1152] -> [8 pairs, 128 partitions, 2 members, 2 subtiles, C]
        xc = xc.reshape(MT // 2, 2, 2, 128, DCOLS).transpose(0, 3, 1, 2, 4)
        x8 = np.ascontiguousarray(xc).astype(ml_dtypes.float8_e4m3)
        maps.append({"x": x8.reshape((MT // 2) * 128, 2, 2, DCOLS)})
    return maps


def _assemble(outs, F64):
    quad = {}
    for c, units in enumerate(ASSIGN):
        o = outs[c].astype(np.float64)
        col = 0
        for (stc, mc, nw) in WINDOWS:
            su = units[stc // 128]
            m0 = mc // 128
            for t in range(nw // 128):
                quad[(su, units[m0 + t])] = o[:, col + t * 128:
                                              col + (t + 1) * 128]
            col += nw
    # exact centering stats in f64
    s_vec = [F64[:, i * 256:(i + 1) * 256].sum(axis=0) for i in range(10)]
    t_vec = [F64[:, i * 256:(i + 1) * 256] @ s_vec[i] for i in range(10)]
    loss = 0.0
    for i in range(10):
        for j in range(i + 1, 10):
            asq = 0.0
            for a in range(2):
                for b in range(2):
                    u, v = 2 * i + a, 2 * j + b
                    q = quad[(u, v)] if (u, v) in quad else quad[(v, u)]
                    asq += float((q * q).sum())
            loss += (asq - (2.0 / N) * float(t_vec[i] @ t_vec[j])
                     + float(s_vec[i] @ s_vec[i]) * float(s_vec[j] @ s_vec[j])
                     / float(N * N))
    loss /= float((N - 1) * (N - 1))
    return np.asarray([loss], np.float32)


def kernel(final_readout, weight, _trace=False):
    X = np.asarray(final_readout, np.float32)
    w = np.asarray(weight, np.float32)
    F64 = (w.astype(np.float64) ** 2) * X.astype(np.float64)
    F32 = F64.astype(np.float32)
    nc = _get_nc()
    res = bass_utils.run_bass_kernel_spmd(
        nc, _in_maps(F32), core_ids=list(range(8)), trace=_trace)
    _CACHE["last_results"] = res
    return _assemble([r["out"] for r in res.results], F64)


# revision 10
# speedup vs baseline: 1.1038x; 1.1038x over previous
"""HSIC pairwise loss kernel for trn2 (8 NeuronCores).

Math: reference builds K_c = (w^2 w^2T) * (E_c E_c^T), M_c = R K_c, and sums
tr(M_i M_j) over i<j. With F_c = w^2 * E_c (row scaling), R the centering
matrix (idempotent):
    tr(R K_i R K_j) = ||G_i^T G_j||_F^2,  G_c = F_c - colmean(F_c)
and with A_ij = F_i^T F_j, s_c = F_c^T 1, t_c = F_c s_c:
    ||G_i^T G_j||_F^2 = ||A_ij||_F^2 - (2/n) t_i.t_j + ||s_i||^2 ||s_j||^2 / n^2

Device work: the 45 A_ij blocks [256,256] (contraction over n=4096) at
half-chunk granularity (20 units of 128 cols). Each core loads 9 units,
pre-scaled by w^2 and cast to fp8e4m3 on host, laid out as 16 k-macrotiles
of 256 rows ([128 partitions, 2 k-subtiles, 1152 cols]). Matmuls run in
fp8 DoubleRow perf mode (256-deep contraction per instruction, double PE
rate), accumulating the pair windows in PSUM over the 16 macrotiles.

Schedule details: the whole per-core input (4.7 MB) is SBUF-resident;
loads are paired (4.6 KB per partition line) and issued alternately from
the Sync and Activation DGE sequencers (descriptor generation is ~0.6us
per DMA and serializes per sequencer). The first macrotile is split so
the PE can start on a 160 KB prefix; dummy self-matmuls warm the PE out
of its low-power state during the initial DMA wait. Output: raw A blocks
cast bf16 (vector engine for the early-stopping windows, activation
engine for the tail) and written by one DMA per DGE sequencer. Host
computes the rank-1 centering corrections (s, t) exactly in float64 and
the final scalar reduction.
"""

import numpy as np
import ml_dtypes
from contextlib import ExitStack

import concourse.bass as bass
import concourse.tile as tile
from concourse import bacc, mybir
from concourse import bass_utils

N = 4096
MT = 16            # k macrotiles of 256 rows (2 DoubleRow subtiles of 128)
UNITS = 9          # half-chunk units per core
DCOLS = UNITS * 128          # 1152 data cols

# 8 cores x 9 ordered units (of 20 half-chunks); covers all 180
# cross-parent half-pairs via the window pattern below (found by search).
# Slot layout: slots 0-3 = "moving" block (cols 0:512), slots 4-8 =
# "stationary" block (cols 512:1152).
ASSIGN = [
    [18, 13, 17, 1, 0, 19, 9, 4, 3],
    [12, 9, 16, 15, 10, 5, 8, 18, 4],
    [2, 7, 14, 11, 5, 17, 12, 9, 18],
    [13, 1, 15, 16, 14, 7, 12, 11, 2],
    [6, 12, 11, 15, 0, 17, 3, 19, 4],
    [3, 19, 0, 6, 14, 2, 9, 16, 5],
    [6, 5, 8, 10, 17, 18, 13, 11, 1],
    [4, 7, 10, 8, 3, 2, 19, 0, 14],
]

# (stationary_col, moving_start_col, n_cols):
# bipartite {slots 4-8} x {slots 0-3} + clique on {0-3}
WINDOWS = [
    (512, 0, 512),
    (640, 0, 512),
    (768, 0, 512),
    (896, 0, 512),
    (1024, 0, 512),
    (0, 128, 384),
    (128, 256, 256),
    (256, 384, 128),
]
# cast engine per window: first VEC_W windows on vector, rest on scalar
VEC_W = 4
OUT_COLS = sum(w[2] for w in WINDOWS)
SPLIT_COL = sum(w[2] for w in WINDOWS[:VEC_W])
WARMUP_MM = 5      # dummy 512-col matmuls to ramp the PE p-state

_CACHE = {}


def _build():
    f32 = mybir.dt.float32
    bf16 = mybir.dt.bfloat16
    f8 = mybir.dt.float8e4
    DR = mybir.MatmulPerfMode.DoubleRow
    nc = bacc.Bacc("TRN2", target_bir_lowering=False, debug=False,
                   num_devices=8)
    # row 128*q + p holds [pair member j, k-subtile s, col c] for F row
    # 512*q + 256*j + 128*s + p
    x = nc.dram_tensor("x", [(MT // 2) * 128, 2, 2, DCOLS], f8,
                       kind="ExternalInput").ap()
    out = nc.dram_tensor("out", [128, OUT_COLS], bf16,
                         kind="ExternalOutput").ap()

    with tile.TileContext(nc) as tc:
        with ExitStack() as ctx:
            xpool = ctx.enter_context(tc.tile_pool(name="xs", bufs=1))
            psum = ctx.enter_context(tc.tile_pool(name="ps", bufs=1,
                                                  space="PSUM"))
            opool = ctx.enter_context(tc.tile_pool(name="o", bufs=1))

            # one PSUM bank per window: a start=True reset zeroes at bank
            # granularity, so windows must not share banks
            ps = []
            for i, (_, _, nw) in enumerate(WINDOWS):
                t = psum.tile([128, nw], f32, tag=f"psb{i}", name=f"psb{i}")
                ps.append((t, 0))

            # PE warmup: dummy DoubleRow matmuls (into window 0's bank,
            # before its accumulation group opens) keep the PE busy and
            # ramping to its fast p-state during the initial DMA wait.
            dsb = xpool.tile([128, 2, 512], f8, name="warm_in")
            nc.vector.memset(dsb[:], 0.0)
            for _ in range(WARMUP_MM):
                nc.tensor.matmul(ps[0][0][:, 0:512], dsb[:, :, 0:128],
                                 dsb[:, :, 0:512], start=True, stop=True,
                                 perf_mode=DR)

            # macrotile 0/1 as separate half-pair tiles (smaller first
            # transfers); pairs 1..7 as full 4.6KB-per-line loads
            t0a = xpool.tile([128, 2, DCOLS], f8, name="t0a")
            t0b = xpool.tile([128, 2, DCOLS], f8, name="t0b")
            nc.sync.dma_start(t0a[:], x[0:128, 0:1, :, :])
            nc.scalar.dma_start(t0b[:], x[0:128, 1:2, :, :])
            pair_tiles = []
            for q in range(1, MT // 2):
                tq = xpool.tile([128, 2, 2, DCOLS], f8, name=f"t{q}")
                eng = nc.sync if q % 2 == 0 else nc.scalar
                eng.dma_start(tq[:], x[q * 128:(q + 1) * 128, :, :, :])
                pair_tiles.append(tq)

            def slices(m, c0, c1):
                """AP for cols [c0:c1) of macrotile m."""
                if m == 0:
                    return t0a[:, :, c0:c1]
                if m == 1:
                    return t0b[:, :, c0:c1]
                return pair_tiles[m // 2 - 1][:, m % 2, :, c0:c1]

            for m in range(MT):
                order = list(range(len(WINDOWS)))
                for wi in order:
                    stc, mc, nw = WINDOWS[wi]
                    pt, po = ps[wi]
                    nc.tensor.matmul(
                        pt[:, po:po + nw],
                        slices(m, stc, stc + 128),
                        slices(m, mc, mc + nw),
                        start=(m == 0),
                        stop=(m == MT - 1),
                        perf_mode=DR,
                    )

            ot = opool.tile([128, OUT_COLS], bf16)
            col = 0
            for wi, (s, mc, nw) in enumerate(WINDOWS):
                pt, po = ps[wi]
                if wi < VEC_W:
                    nc.vector.tensor_copy(ot[:, col:col + nw],
                                          pt[:, po:po + nw])
                else:
                    nc.scalar.copy(ot[:, col:col + nw], pt[:, po:po + nw])
                col += nw
            nc.sync.dma_start(out[:, 0:SPLIT_COL], ot[:, 0:SPLIT_COL])
            nc.scalar.dma_start(out[:, SPLIT_COL:OUT_COLS],
                                ot[:, SPLIT_COL:OUT_COLS])
    nc.compile()
    return nc


def _get_nc():
    if "nc" not in _CACHE:
        _CACHE["nc"] = _build()
    return _CACHE["nc"]


def _in_maps(F32):
    maps = []
    for units in ASSIGN:
        xc = np.concatenate([F32[:, u * 128:(u + 1) * 128] for u in units],
                            axis=1)
        # [4096, 1152] -> [8 pairs, 128 partitions, 2 members, 2 subtiles, C]
        xc = xc.reshape(MT // 2, 2, 2, 128, DCOLS).transpose(0, 3, 1, 2, 4)
        x8 = np.ascontiguousarray(xc).astype(ml_dtypes.float8_e4m3)
        maps.append({"x": x8.reshape((MT // 2) * 128, 2, 2, DCOLS)})
    return maps


def _assemble(outs, F64):
    quad = {}
    for c, units in enumerate(ASSIGN):
        o = outs[c].astype(np.float64)
        col = 0
        for (stc, mc, nw) in WINDOWS:
            su = units[stc // 128]
            m0 = mc // 128
            for t in range(nw // 128):
                quad[(su, units[m0 + t])] = o[:, col + t * 128:
                                              col + (t + 1) * 128]
            col += nw
    # exact centering stats in f64
    s_vec = [F64[:, i * 256:(i + 1) * 256].sum(axis=0) for i in range(10)]
    t_vec = [F64[:, i * 256:(i + 1) * 256] @ s_vec[i] for i in range(10)]
    loss = 0.0
    for i in range(10):
        for j in range(i + 1, 10):
            asq = 0.0
            for a in range(2):
                for b in range(2):
                    u, v = 2 * i + a, 2 * j + b
                    q = quad[(u, v)] if (u, v) in quad else quad[(v, u)]
                    asq += float((q * q).sum())
            loss += (asq - (2.0 / N) * float(t_vec[i] @ t_vec[j])
                     + float(s_vec[i] @ s_vec[i]) * float(s_vec[j] @ s_vec[j])
                     / float(N * N))
    loss /= float((N - 1) * (N - 1))
    return np.asarray([loss], np.float32)


def kernel(final_readout, weight, _trace=False):
    X = np.asarray(final_readout, np.float32)
    w = np.asarray(weight, np.float32)
    F64 = (w.astype(np.float64) ** 2) * X.astype(np.float64)
    F32 = F64.astype(np.float32)
    nc = _get_nc()
    res = bass_utils.run_bass_kernel_spmd(
        nc, _in_maps(F32), core_ids=list(range(8)), trace=_trace)
    _CACHE["last_results"] = res
    return _assemble([r["out"] for r in res.results], F64)
